# revision 11
# baseline (speedup 1.0000x reference)
"""GAT layer (nn_GATLayer) Trainium2 Bass kernel, 8-core SPMD. v3.

Math: the reference GAT softmax factorizes. scores[n,h,m] =
exp(s_src[n,h]) * exp(s_dst[m,h] + b_attn[h]) * adj_sl[n,m], and the
row-normalization cancels the exp(s_src) factor (EPS=1e-10 is ~1e-11
relative - far below fp32 noise). So with

    e[m,h]  = exp(features[m] @ Wa_dst[h] + b_attn[h])
    ft[m,:] = features[m] @ W_lin.T + b_lin          (128 cols, 2 heads x 64)
    G[m,:]  = [e[m,0]*ft[m,0:64], e[m,1]*ft[m,64:128], e[m,0], e[m,1]]

the whole layer is ONE big matmul  Y = adj_sl @ G  ([8192, 130]) plus
    out[n, h*64+j] = elu(Y[n, h*64+j] / Y[n, 128+h]).

v3 layout: the host pre-transposes each core's adjacency row-slab and
pre-casts it to fp8 E4M3 (0/1/2 are exact), so the device reads
adjT [8192, 1024] fp8 with the contraction index m on partitions:
  - HBM traffic is 1/4 of the fp32 slab (8 MB/core)
  - no PE transposes, no PSUM transpose drains, no cast ops
  - fp8 stationary weights get the 4x fast-weight-load path, so the
    512 accumulation matmuls stream back-to-back at ~N cols/cycle
Main loop: stream 4x 2MB m-band groups; for each of 64 m-bands, 8
matmuls (stationary = adjT block [128m x 128n] fp8, moving = G[mt]
[128m x 130] bf16) accumulate into 8 PSUM banks (two [128,4,512]
bank-aligned accumulator tiles), one bank per 128-row destination
group. Single bf16 G: measured max rel err vs fp64 oracle is 2.9e-3,
dominated by the bf16 feature matmul, so an fp32-ness hi/lo split of
G buys nothing. Preproc and epilogue are batched into a handful of
wide strided ops rather than per-tile chains.

Sharding: row-shard destination nodes n across 8 cores (1024 rows
each); G / features / weights replicated; no cross-core reduction.
"""

import numpy as np
import ml_dtypes

import concourse.bass as bass
import concourse.mybir as mybir
import concourse.tile as tile
from concourse import bacc
from concourse.bass_utils import run_bass_kernel_spmd

F32 = mybir.dt.float32
BF16 = mybir.dt.bfloat16
FP8 = mybir.dt.float8e4
NP_FP8 = ml_dtypes.float8_e4m3
NP_BF16 = ml_dtypes.bfloat16

N = 8192
IN_DIM = 64
OUT_DIM = 64
HEADS = 2
NCORES = 8
ROWS = N // NCORES          # 1024 destination rows per core
NT = ROWS // 128            # 8 n-tiles per core
MT = N // 128               # 64 m-tiles (full source dim)
C = HEADS * OUT_DIM + HEADS  # 130 columns of G
FT65 = IN_DIM + 1           # features_T plus a ones-row (bias folding)
# m-bands per DMA group: two 4 MB transfers. Fewer, larger DMAs win
# here: each extra dma_start costs far more scheduling overhead than
# the ~3 us of pipeline fill a finer staircase would recover, and the
# 3-deep band ring already overlaps the next rep's first transfer.
GROUPS = (32, 32)
PMM = 12                    # preproc matmuls drained per PSUM slot (3/bank)
AF = mybir.ActivationFunctionType


def build_program(reps: int = 1):
    """Trace + compile the SPMD program. reps>1 repeats the whole
    pipeline (for wall-clock slope timing); outputs are overwritten."""
    nc = bacc.Bacc("TRN2", target_bir_lowering=False, debug=False,
                   num_devices=NCORES)

    adjt = nc.dram_tensor("adjt", [N, ROWS], FP8, kind="ExternalInput").ap()
    ft65 = nc.dram_tensor("ft65", [FT65, N], BF16, kind="ExternalInput").ap()
    wcat = nc.dram_tensor("wcat", [FT65, C], BF16, kind="ExternalInput").ap()
    out = nc.dram_tensor("out", [ROWS, HEADS * OUT_DIM], F32,
                         kind="ExternalOutput").ap()

    with tile.TileContext(nc) as tc:
        with tc.tile_pool(name="const", bufs=1) as const, \
             tc.tile_pool(name="gpool", bufs=1) as gpool, \
             tc.tile_pool(name="band_p", bufs=3) as band_p, \
             tc.tile_pool(name="ep", bufs=1) as ep, \
             tc.tile_pool(name="ps", bufs=2, space="PSUM") as ps_pool:

            ft_sb = const.tile([FT65, N], BF16)
            nc.sync.dma_start(out=ft_sb, in_=ft65)
            wc_sb = const.tile([FT65, C], BF16)
            nc.sync.dma_start(out=wc_sb, in_=wcat)

            for _rep in range(reps):
                # ---- preproc: ft/e staging in fp32, 12 matmuls per drain ----
                pp = gpool.tile([128, MT, C], F32, name="pp")
                for k0 in range(0, MT, PMM):
                    kn = min(PMM, MT - k0)
                    psg = ps_pool.tile([128, 4, 512], F32, name="psg",
                                       tag="ps")
                    for j in range(kn):
                        mt = k0 + j
                        nc.tensor.matmul(
                            psg[:, j // 3, (j % 3) * C:(j % 3) * C + C],
                            ft_sb[:, mt * 128:(mt + 1) * 128],
                            wc_sb, start=True, stop=True)
                    # drain: one strided copy per run of full 3-mt banks,
                    # plus one for a partial tail bank; alternate DVE and
                    # ACT so consecutive chunk drains overlap
                    eng = nc.vector if (k0 // PMM) % 2 == 0 else nc.scalar
                    cop = (eng.tensor_copy if eng is nc.vector
                           else nc.scalar.copy)
                    nfull, rem = divmod(kn, 3)
                    if nfull:
                        src = bass.AP(tensor=psg.tensor, offset=psg.offset,
                                      ap=[list(psg.ap[0]), [512, nfull],
                                          [1, 3 * C]])
                        dst = bass.AP(tensor=pp.tensor,
                                      offset=pp.offset + k0 * C,
                                      ap=[list(pp.ap[0]), [3 * C, nfull],
                                          [1, 3 * C]])
                        cop(dst, src)
                    if rem:
                        src = bass.AP(tensor=psg.tensor,
                                      offset=psg.offset + nfull * 512,
                                      ap=[list(psg.ap[0]), [1, rem * C]])
                        dst = bass.AP(tensor=pp.tensor,
                                      offset=pp.offset + (k0 + nfull * 3) * C,
                                      ap=[list(pp.ap[0]), [1, rem * C]])
                        cop(dst, src)
                # e = exp(s_dst + b), written bf16 straight into G; split
                # in two half-batches so G[m-tiles of DMA group 0] is
                # ready when the first adjacency transfer lands instead
                # of after the full-width multiply
                g_sb = gpool.tile([128, MT, C], BF16, name="g_sb")
                for a, b in ((0, 3 * PMM), (3 * PMM, MT)):
                    nc.scalar.activation(g_sb[:, a:b, 128:130],
                                         pp[:, a:b, 128:130], AF.Exp)
                    # G[:, :, h*64:..] = pp * e  (free-step-0 bcast of e)
                    for h in range(HEADS):
                        e_rep = bass.AP(tensor=g_sb.tensor,
                                        offset=g_sb.offset + a * C + 128 + h,
                                        ap=[list(g_sb.ap[0]), [C, b - a],
                                            [0, OUT_DIM]])
                        nc.vector.tensor_mul(
                            g_sb[:, a:b, h * 64:(h + 1) * 64],
                            pp[:, a:b, h * 64:(h + 1) * 64], e_rep)

                # ---- main: stream adjT m-bands, accumulate 8 PSUM banks ----
                acc = [ps_pool.tile([128, 4, 512], F32, name=f"acc{i}",
                                    tag="ps") for i in range(2)]
                mt0 = 0
                for mg_sz in GROUPS:
                    grp = band_p.tile([128, mg_sz, ROWS], FP8, name="grp",
                                      tag="grp")
                    src = adjt[mt0 * 128:(mt0 + mg_sz) * 128, :]
                    nc.sync.dma_start(
                        out=grp, in_=src.rearrange("(b p) n -> p b n", p=128))
                    for b in range(mg_sz):
                        mt = mt0 + b
                        for g in range(NT):
                            nc.tensor.matmul(
                                acc[g // 4][:, g % 4, 0:C],
                                grp[:, b, g * 128:(g + 1) * 128],
                                g_sb[:, mt, :],
                                start=(mt == 0), stop=(mt == MT - 1))
                    mt0 += mg_sz

                # ---- epilogue: normalize + ELU + store (batched) ----
                y = ep.tile([128, NT, C], F32, name="y")
                nc.scalar.copy(y[:, 0:4, :], acc[0][:, :, 0:C])
                nc.vector.tensor_copy(y[:, 4:8, :], acc[1][:, :, 0:C])
                r2 = ep.tile([128, NT, HEADS], F32, name="r2")
                nc.vector.reciprocal(r2, y[:, :, 128:130])
                o1 = ep.tile([128, NT, 128], F32, name="o1")
                for h in range(HEADS):
                    r_rep = bass.AP(tensor=r2.tensor, offset=r2.offset + h,
                                    ap=[list(r2.ap[0]), [HEADS, NT],
                                        [0, OUT_DIM]])
                    nc.vector.tensor_mul(o1[:, :, h * 64:(h + 1) * 64],
                                         y[:, :, h * 64:(h + 1) * 64], r_rep)
                mn = ep.tile([128, NT, 128], F32, name="mn")
                nc.vector.tensor_scalar_min(mn, o1, 0.0)
                ex = ep.tile([128, NT, 128], F32, name="ex")
                nc.scalar.activation(ex, mn, AF.Exp)
                # elu = (x - min(x,0)) + exp(min(x,0)) - 1
                nc.vector.tensor_sub(o1, o1, mn)
                nc.vector.tensor_add(o1, o1, ex)
                nc.vector.tensor_scalar_add(o1, o1, -1.0)
                nc.sync.dma_start(
                    out=out.rearrange("(g p) c -> p g c", p=128), in_=o1)

    nc.compile()
    return nc


def make_in_maps(adj, features, W_attn, b_attn, W_lin, b_lin):
    """Host-side input marshalling: per-core transposed fp8 adjacency
    slabs (+ self-loop diagonal bump), transposed/concatenated small
    operands. adj values are 0/1/2 - exact in fp8 E4M3."""
    adj = np.asarray(adj, dtype=np.float32)
    features = np.asarray(features, dtype=np.float32)
    W_attn = np.asarray(W_attn, dtype=np.float32)
    b_attn = np.asarray(b_attn, dtype=np.float32)
    W_lin = np.asarray(W_lin, dtype=np.float32)
    b_lin = np.asarray(b_lin, dtype=np.float32)

    ft65 = np.concatenate([features.T.astype(NP_BF16),
                           np.ones((1, N), NP_BF16)], axis=0)
    ft65 = np.ascontiguousarray(ft65)
    wcat = np.zeros((FT65, C), NP_BF16)
    wcat[:IN_DIM, 0:HEADS * OUT_DIM] = W_lin.T.astype(NP_BF16)
    wcat[:IN_DIM, HEADS * OUT_DIM:] = W_attn[:, IN_DIM:].T.astype(NP_BF16)
    wcat[IN_DIM, 0:HEADS * OUT_DIM] = b_lin.astype(NP_BF16)
    wcat[IN_DIM, HEADS * OUT_DIM:] = b_attn.astype(NP_BF16)

    adj_f8 = adj.astype(NP_FP8)  # 0/1 exact
    in_maps = []
    r = np.arange(ROWS)
    for c in range(NCORES):
        slab = np.ascontiguousarray(adj_f8[c * ROWS:(c + 1) * ROWS, :].T)
        slab[c * ROWS + r, r] += np.asarray(1.0, NP_FP8)     # self-loops
        in_maps.append({"adjt": slab, "ft65": ft65, "wcat": wcat})
    return in_maps


_CACHED = {}


def _get_program(reps=1):
    if reps not in _CACHED:
        _CACHED[reps] = build_program(reps)
    return _CACHED[reps]


def run_on_device(in_maps, reps=1, **kw):
    nc = _get_program(reps)
    res = run_bass_kernel_spmd(nc, in_maps, core_ids=list(range(NCORES)), **kw)
    return res


def kernel(adj, features, W_attn, b_attn, W_lin, b_lin):
    in_maps = make_in_maps(adj, features, W_attn, b_attn, W_lin, b_lin)
    res = run_on_device(in_maps, reps=1)
    return np.concatenate([res.results[c]["out"] for c in range(NCORES)],
                          axis=0)
